# revision 46
# baseline (speedup 1.0000x reference)
"""Multi-head attention (nonstandard softmax normalization) on 8 Trainium2 cores.

Reference computation (B=4, E=1024, S=1024, H=16, HS=64):
  per (b, h):  q = Wq[h] @ Q_h,  k = Wk[h] @ K_h,  v = Wv[h] @ V_h   (feature-first [HS, S])
               pre[s,t] = q[:,s]. k[:,t] / 8
               e = exp(pre);  denom[t] = sum_u e[t,u];  post[s,t] = e[s,t] / denom[t]
               out_h = v @ post.T                                     ([HS, S])
  out = concat_h(out_h);  result[b] = Wo @ out[b]
Sharding: core c -> (b = c//2, head-group hg = c%2 of 8 heads); host sums the
two partial Wo products per batch.

The kernel is ACT(exp)-roofline bound: 8 heads x S^2 exps per core = 64
ACTIVATE instructions of N=1024 ~= 77us dense.  Everything is organized to
keep the exp stream dense:
 - per pair (2 heads), QK^T is computed transposed in [128,1024] fp32 PSUM
   tiles (tag "pqk", 3 bufs = 6 banks) so exp reads N=1024 chunks while the
   next chunk's matmuls run (pipeline depth 1.5 chunks).
 - the two heads' QK matmuls are K=64 row-tiles (base_partition 0/64) that
   run concurrently on the PE; AV uses col-tiled M=64 pairs.
 - the denominator (partition-dim sums of E) accumulates with lag-2 behind
   exp into ONE packed PSUM bank (h0 cols 0:256, h1 cols 256:512; per-element
   has_written semantics make the shared bank safe).
 - denom free-dim row -> per-partition recip via one strided DVE copy and a
   single SBUF->SBUF scatter DMA (no DRAM round trip), then one fused
   reciprocal + one broadcast multiply folds recip(denom) into v^T.
 - projections/v^T for pairs 1-3 are emitted as filler inside pair 0's QK
   stream so the first exp starts as early as possible.
 - the Wo phase recycles the (dead) "pqk" PSUM slots for dense accumulation.
"""

import os
import sys
import types

import numpy as np

import concourse.bass as bass
import concourse.mybir as mybir
import concourse.tile as tile
from contextlib import ExitStack

B, E, S_FULL, H = 4, 1024, 1024, 16
HS = 64
N_CORES = 8
HEADS_PER_CORE = H // 2          # 8: head-group per core
N_PAIRS_FULL = HEADS_PER_CORE // 2  # 4

_f32 = mybir.dt.float32


def _install_ntff_shim():
    """Register the axon NTFF profile hook if the image's antenv lacks it."""
    try:
        import antenv.axon_hooks  # noqa: F401
        return
    except ImportError:
        pass
    try:
        import antenv
        from trn_agent_boot.trn_boot import _ntff_profile_via_ctypes
    except ImportError:
        return
    mod = types.ModuleType("antenv.axon_hooks")
    mod._hook = None

    def set_axon_ntff_profile_hook(h):
        mod._hook = h

    def get_axon_ntff_profile_hook():
        return mod._hook

    mod.set_axon_ntff_profile_hook = set_axon_ntff_profile_hook
    mod.get_axon_ntff_profile_hook = get_axon_ntff_profile_hook
    sys.modules["antenv.axon_hooks"] = mod
    antenv.axon_hooks = mod
    for so in ("/opt/axon/libaxon_pjrt.so",):
        if os.path.exists(so):
            try:
                mod._hook = _ntff_profile_via_ctypes(so)
            except Exception:
                mod._hook = None
            break


def _install_drain_patch():
    """Work around this toolchain's walrus rejecting sem waits on Drain.

    TileContext's final drain carries end-of-kernel semaphore waits inline;
    this walrus build encodes Drain as NEURON_ISA_TPB_CTRL_NO_STRUCT and
    fails codegen ("Too many sync wait commands") for ANY inline wait.
    Equivalent semantics: emit the waits as standalone sync-engine wait
    instructions and leave the Drain bare.
    """
    if getattr(tile.TileContext, "_drain_patch_installed", False):
        return
    from concourse.vector_clock import ScopedClock

    def _patched_drain_and_barrier(self, tick_clock, wait_clock):
        drain_inst = self.nc.sync.drain()
        wait_clock.add_sem_waits(
            drain_inst.ins, ScopedClock({None: tick_clock.global_clock})
        )
        si = drain_inst.ins.sync_info
        waits = list(si.on_wait) if si is not None else []
        if waits:
            drain_inst.ins.sync_info = mybir.SyncInfo(
                on_wait=[], on_update=list(si.on_update) if si.on_update else []
            )
            by_name = (
                {h.name: h for h in self.sems.allocated().values()}
                if self.sems is not None else {}
            )
            for w in waits:
                sem = by_name.get(w.ant_name)
                assert sem is not None, f"unknown drain-wait sem: {w.ant_name}"
                assert w.wait_mode == "sem-ge-imm", w
                self.nc.sync.wait_ge(sem, w.wait_value)
        self.nc.all_engine_barrier()
        assert self.sems is not None
        popped = self.nc._tile_sem_poison_stack.pop()
        assert popped is self._sem_poison
        self.nc.clear_and_free_semaphores(list(self.sems.allocated().values()))
        self.nc.all_engine_barrier()

    tile.TileContext._drain_and_barrier = _patched_drain_and_barrier

    # Same walrus limitation, general form: at most ONE inline sem wait per
    # instruction; hoist all but the last onto EventSemaphore carriers.
    orig_add = tile.TileContext._add_instruction

    def _split_add_instruction(self, inst):
        si = inst.sync_info
        if si is not None and si.on_wait and len(si.on_wait) > 1:
            waits = list(si.on_wait)
            for w in waits[:-1]:
                ev = mybir.InstEventSemaphore(
                    name=self.nc.get_next_instruction_name(),
                    engine=inst.engine,
                    sync_info=mybir.SyncInfo(on_wait=[w], on_update=[]),
                )
                orig_add(self, ev)
            inst.sync_info = mybir.SyncInfo(
                on_wait=[waits[-1]],
                on_update=list(si.on_update) if si.on_update else [],
            )
        orig_add(self, inst)

    tile.TileContext._add_instruction = _split_add_instruction
    tile.TileContext._drain_patch_installed = True


def build_core_kernel(S=1024, n_pairs=4, e_out=1024, mm_dt=mybir.dt.float16,
                      e_dt=mybir.dt.float16, sbuf_dma=True):
    """Build the per-core Bass program (SPMD: same program on all cores)."""
    _install_drain_patch()
    C = S // 128            # t-chunks (chunk-contiguous: t = c*128 + p)
    NT = min(512, S)        # matmul moving free-dim tile
    NS = S // NT            # s-tiles (2)
    S4 = S // 4             # denom col-group width (256)
    EC = e_out // 128       # output e-chunks
    FP = n_pairs * 128      # feature rows handled by this core
    f32 = _f32
    in_dt = mm_dt

    nc = bass.Bass()
    q_rows = nc.declare_dram_parameter("q_rows", [FP, S], in_dt, isOutput=False)
    k_rows = nc.declare_dram_parameter("k_rows", [FP, S], in_dt, isOutput=False)
    v_rows = nc.declare_dram_parameter("v_rows", [FP, S], in_dt, isOutput=False)
    wqT = nc.declare_dram_parameter("wqT", [n_pairs, 128, 128], in_dt, isOutput=False)
    wkT = nc.declare_dram_parameter("wkT", [n_pairs, 128, 128], in_dt, isOutput=False)
    wvT = nc.declare_dram_parameter("wvT", [n_pairs, 128, 128], in_dt, isOutput=False)
    woT = nc.declare_dram_parameter("woT", [FP, e_out], in_dt, isOutput=False)
    out_part = nc.declare_dram_parameter("out_part", [e_out, S], mm_dt, isOutput=True)

    Exp = mybir.ActivationFunctionType.Exp
    Mult = mybir.AluOpType.mult

    with tile.TileContext(nc) as tc, ExitStack() as ctx:
        raws = ctx.enter_context(tc.tile_pool(name="raws", bufs=6))
        wop = ctx.enter_context(tc.tile_pool(name="wop", bufs=1))
        consts = ctx.enter_context(tc.tile_pool(name="consts", bufs=1))
        qks = ctx.enter_context(tc.tile_pool(name="qks", bufs=2))
        vts = ctx.enter_context(tc.tile_pool(name="vts", bufs=n_pairs))
        Epool = ctx.enter_context(tc.tile_pool(name="Epool", bufs=6))
        outp = ctx.enter_context(tc.tile_pool(name="outp", bufs=1))
        rcp = ctx.enter_context(tc.tile_pool(name="rcp", bufs=2))
        dstp = ctx.enter_context(tc.tile_pool(name="dstp", bufs=2))
        wostp = ctx.enter_context(tc.tile_pool(name="wostp", bufs=3))
        dram = ctx.enter_context(tc.tile_pool(name="dscratch", bufs=4, space="DRAM"))
        # one PSUM pool, 8 banks exactly:
        #   tag pqk : 3 x [128,1024] f32 = 6 banks (QK->exp pipeline)
        #   tag dps : 1 x [128, 512] f32 = 1 bank  (packed denominator)
        #   tag avp : 1 x [128, 512] f32 = 1 bank  (AV accum + transients)
        psum = ctx.enter_context(tc.tile_pool(name="psum", bufs=3, space="PSUM"))

        ones = consts.tile([128, 1], e_dt, tag="ones")
        nc.vector.memset(ones, 1.0)
        zeros = consts.tile([128, NT], mm_dt, tag="zeros")
        nc.vector.memset(zeros, 0.0)
        # HAM warm-up: dummy matmuls keep the PE busy while the first input
        # DMAs land, so the first real matmuls run at 2.4 GHz, not 1.2.
        warm_ps = psum.tile([128, NT], f32, tag="dps", bufs=1, name="warm_ps")
        for _ in range(12):
            nc.tensor.matmul(warm_ps, lhsT=zeros[:, :128], rhs=zeros,
                             start=True, stop=True)
        wq_sb = consts.tile([128, n_pairs, 128], in_dt, tag="wq")
        wk_sb = consts.tile([128, n_pairs, 128], in_dt, tag="wk")
        wv_sb = consts.tile([128, n_pairs, 128], in_dt, tag="wv")
        nc.sync.dma_start(out=wq_sb, in_=wqT.rearrange("r p m -> p r m"))
        nc.sync.dma_start(out=wk_sb, in_=wkT.rearrange("r p m -> p r m"))
        nc.sync.dma_start(out=wv_sb, in_=wvT.rearrange("r p m -> p r m"))

        q_all = qks.tile([128, n_pairs, S], mm_dt, tag="qall")
        k_all = qks.tile([128, n_pairs, S], mm_dt, tag="qall")
        out_all = outp.tile([128, n_pairs, S], mm_dt, tag="outall")

        vt_tiles = [None] * n_pairs
        raw_qk = {}
        raw_v = {}

        def load_qk(pr):
            qr = raws.tile([128, S], in_dt, tag="raw", name=f"qr_{pr}")
            kr = raws.tile([128, S], in_dt, tag="raw", name=f"kr_{pr}")
            # pair 0 gates the first exp: split its loads across more DMA
            # queues so the first projection starts sooner
            nsl = 4 if pr == 0 else NS
            w = S // nsl
            for st in range(nsl):
                sl = slice(st * w, (st + 1) * w)
                nc.sync.dma_start(out=qr[:, sl],
                                  in_=q_rows[pr * 128:(pr + 1) * 128, sl])
                nc.sync.dma_start(out=kr[:, sl],
                                  in_=k_rows[pr * 128:(pr + 1) * 128, sl])
            raw_qk[pr] = (qr, kr)

        def load_v(pr):
            vr = raws.tile([128, S], in_dt, tag="raw", name=f"vr_{pr}")
            nc.sync.dma_start(out=vr, in_=v_rows[pr * 128:(pr + 1) * 128, :])
            raw_v[pr] = vr

        _tcnt = [0]

        def trans_ps(name, tag="avp"):
            """Transient [128,512] PSUM tile; mid-stream transients share
            the 'avp' slot (pair 0's upfront proj can use free pqk slots)."""
            _tcnt[0] += 1
            bufs = None if tag == "pqk" else 1
            return psum.tile([128, NT], f32, tag=tag, bufs=bufs,
                             name=f"{name}_{_tcnt[0]}")

        def proj_qk(pr, which=None, tag="avp"):
            """Project q (which=0), k (which=1), or both into q_all/k_all."""
            qr, kr = raw_qk[pr]
            plan = ((qr, q_all, wq_sb), (kr, k_all, wk_sb))
            if which is not None:
                plan = (plan[which],)
            for src, dst, wt in plan:
                for st in range(NS):
                    ps = trans_ps(f"pj_{pr}_{st}", tag=tag)
                    nc.tensor.matmul(
                        ps,
                        lhsT=wt[:, pr, :],
                        rhs=src[:, st * NT:(st + 1) * NT],
                        start=True, stop=True,
                    )
                    nc.vector.tensor_copy(
                        out=dst[:, pr, st * NT:(st + 1) * NT], in_=ps)
            if which in (None, 1):
                raw_qk.pop(pr)

        def proj_v(pr, half):
            """v^T (transposed, two heads side by side) for 4 c-chunks."""
            vr = raw_v[pr]
            if vt_tiles[pr] is None:
                vt_tiles[pr] = vts.tile([128, C, 128], e_dt, tag="vt",
                                        name=f"vt_{pr}")
            vt = vt_tiles[pr]
            vrc = vr.rearrange("p (c t) -> p c t", c=C)
            c0 = half * (C // 2)
            ps = trans_ps(f"pv_{pr}_{half}")
            for j in range(C // 2):
                nc.tensor.matmul(
                    ps[:, j * 128:(j + 1) * 128],
                    lhsT=vrc[:, c0 + j, :],
                    rhs=wv_sb[:, pr, :],
                    start=True, stop=True,
                )
            nc.vector.tensor_copy(
                out=vt[:, c0:c0 + C // 2, :], in_=ps)
            if half == 1:
                raw_v.pop(pr)

        woT_sb = wop.tile([128, n_pairs, e_out], in_dt, tag="woT")

        # ---- per-pair phase-3 pieces ----
        Es_tiles = {}
        dps_tiles = {}

        def emit_qk_chunk(pr, c):
            kc = [
                k_all[64 * hh:64 * hh + 64, pr, :].rearrange(
                    "p (c t) -> p c t", c=C)
                for hh in (0, 1)
            ]
            pst = [psum.tile([128, S], f32, tag="pqk",
                             name=f"pqk_{pr}_{c}_{i}") for i in (0, 1)]
            def qk_mm(st, hh):
                nc.tensor.matmul(
                    pst[hh][:, st * NT:(st + 1) * NT],
                    lhsT=kc[hh][:, c, :],
                    rhs=q_all[64 * hh:64 * hh + 64, pr,
                              st * NT:(st + 1) * NT],
                    start=True, stop=True,
                    tile_position=(64 * hh, 0),
                    skip_group_check=True,
                )

            def qk_exp(hh):
                # E[t, s] = exp(preT[t, s] / 8)
                nc.scalar.activation(
                    out=Es_tiles[pr][hh][:, c, :], in_=pst[hh][:],
                    func=Exp, scale=0.125)

            # h0's exp is emitted as soon as its two matmuls are done so
            # ACT never waits on h1's last matmul
            qk_mm(0, 0)
            qk_mm(0, 1)
            qk_mm(1, 0)
            qk_exp(0)
            qk_mm(1, 1)
            qk_exp(1)

        def emit_denom_chunk(pr, c, hh):
            """Accumulate denom over chunk c into the packed dps bank.

            Only h0's c0 matmuls carry start=True: the hardware clear of the
            has_written bits is bank-wide, so a second start would wipe h0's
            accumulation.  h1's c0 (flags=0) overwrites because the bank-wide
            clear left its region's bits unset; the caller staggers h1 one
            chunk behind h0 so its writes cannot race the clear.
            """
            if pr not in dps_tiles:
                dps_tiles[pr] = psum.tile([128, NT], f32, tag="dps", bufs=1,
                                          name=f"dps_{pr}")
            dps = dps_tiles[pr]
            for q4 in range(4):
                nc.tensor.matmul(
                    dps[32 * q4:32 * q4 + 1, hh * S4:(hh + 1) * S4],
                    lhsT=ones,
                    rhs=Es_tiles[pr][hh][:, c, q4 * S4:(q4 + 1) * S4],
                    start=(c == 0 and hh == 0), stop=(c == C - 1),
                    tile_position=(0, 32 * q4),
                    skip_group_check=True,
                )

        def emit_recip_scale(pr):
            """denom rows -> per-partition recip, folded into v^T.

            Processed per head half so h0's DRAM round trip (and the vt
            scale of its half) starts one exp earlier than h1's.
            """
            dps = dps_tiles.pop(pr)
            dstage = dstp.tile([128, NT], f32, tag="dstage",
                               name=f"dstage_{pr}")
            rcrs = [rcp.tile([128, C], f32, tag=f"rcr{hh}",
                             name=f"rcr_{pr}_{hh}") for hh in (0, 1)]
            rcs = [rcp.tile([128, C], f32, tag=f"rc{hh}",
                            name=f"rc_{pr}_{hh}") for hh in (0, 1)]
            vt = vt_tiles[pr]
            for hh in (0, 1):
                sl = slice(hh * S4, (hh + 1) * S4)
                # full-partition copy (rows off the 32-grid are unread
                # garbage); one DVE op instead of four row copies
                nc.vector.tensor_copy(out=dstage[:, sl], in_=dps[:, sl])
                flat = dstage.rearrange("(a b) f -> a b f", b=32)[:, 0, :]
                scr = dram.tile([S], f32, tag=f"scr{hh}",
                                name=f"scr_{pr}_{hh}")
                nc.sync.dma_start(
                    out=scr.rearrange("(a f) -> a f", a=4),
                    in_=flat[:, sl])
                nc.sync.dma_start(
                    out=rcrs[hh],
                    in_=scr.rearrange("(c p) -> p c", p=128))
                nc.vector.reciprocal(out=rcs[hh], in_=rcrs[hh])
                nc.vector.tensor_tensor(
                    out=vt[:, :, 64 * hh:64 * hh + 64],
                    in0=vt[:, :, 64 * hh:64 * hh + 64],
                    in1=rcs[hh][:, :, None].to_broadcast((128, C, 64)),
                    op=Mult,
                )

        avp_tiles = {}

        def emit_av_st(pr, st, grp=None, tag="avp"):
            """AV accumulation; grp splits the 8 chunks into quarters so the
            PE burst per emission slot stays under ~1us."""
            key = (pr, st)
            if key not in avp_tiles:
                avp_tiles[key] = psum.tile([128, NT], f32, tag=tag, bufs=1,
                                           name=f"avp_{pr}_{st}")
            avp = avp_tiles[key]
            cr = range(C) if grp is None else range(
                grp * (C // 4), (grp + 1) * (C // 4))
            for c in cr:
                for hh in (0, 1):
                    nc.tensor.matmul(
                        avp[64 * hh:64 * hh + 64, :],
                        lhsT=vt_tiles[pr][:, c, 64 * hh:64 * hh + 64],
                        rhs=Es_tiles[pr][hh][:, c, st * NT:(st + 1) * NT],
                        start=(c == 0), stop=(c == C - 1),
                        tile_position=(0, 64 * hh),
                        skip_group_check=True,
                    )
            if grp is None or grp == 3:
                avp_tiles.pop(key)
                nc.vector.tensor_copy(
                    out=out_all[:, pr, st * NT:(st + 1) * NT], in_=avp)

        # ---- emission schedule ----
        # input DMAs for pair 0 first, then the rest (raws pool depth 6
        # naturally paces the prefetch)
        load_qk(0)
        load_v(0)
        load_qk(1)
        load_v(1)
        # the 1 MB woT transfer queues only after the startup-critical loads
        nc.sync.dma_start(out=woT_sb, in_=woT.rearrange("(f p) e -> p f e", p=128))
        load_qk(2)
        load_v(2)
        load_qk(3)
        load_v(3)

        proj_qk(0, tag="pqk")

        # per-stream filler thunks, indexed by chunk slot.  Each slot's PE
        # work is kept under ~1us so QK chunks (and thus the exp stream)
        # never fall behind; AV bursts are split in half and kept away from
        # the stream boundary.
        def finish_prev(p):
            # pair p's denom leftovers (h0 lag-2, h1 lag-3), then its
            # recip/scale; emitted one slot into the next stream so the
            # exps they wait on are already done when they hit the PE.
            emit_denom_chunk(p, C - 2, 0)
            emit_denom_chunk(p, C - 1, 0)
            emit_denom_chunk(p, C - 3, 1)
            emit_denom_chunk(p, C - 2, 1)
            emit_denom_chunk(p, C - 1, 1)
            emit_recip_scale(p)

        def stream_slots(pr):
            if pr == 0:
                return {
                    0: [lambda: proj_v(0, 0)],
                    1: [lambda: proj_v(0, 1)],
                    2: [lambda: proj_qk(1, 0)],
                    3: [lambda: proj_qk(1, 1)],
                    5: [lambda: proj_qk(2, 0)],
                    7: [lambda: proj_qk(2, 1)],
                }
            p = pr - 1
            slots = {
                0: ([lambda: emit_av_st(p - 1, 1, 3)] if pr >= 2 else []),
                1: [lambda: finish_prev(p)],
                2: [lambda: proj_v(pr, 0)],
                3: [lambda: proj_v(pr, 1)],
                4: [lambda: emit_av_st(p, 0, 0), lambda: emit_av_st(p, 0, 1)],
                5: [lambda: emit_av_st(p, 0, 2), lambda: emit_av_st(p, 0, 3)],
                6: [lambda: emit_av_st(p, 1, 0), lambda: emit_av_st(p, 1, 1)],
                7: [lambda: emit_av_st(p, 1, 2)],
            }
            if pr == 1:
                slots[7].append(lambda: proj_qk(3, 0))
            if pr == 2:
                slots[7].append(lambda: proj_qk(3, 1))
            return slots

        for pr in range(n_pairs):
            Es_tiles[pr] = (
                Epool.tile([128, C, S], e_dt, tag="E", name=f"E0_{pr}"),
                Epool.tile([128, C, S], e_dt, tag="E", name=f"E1_{pr}"),
            )
            slots = stream_slots(pr)
            for c in range(C):
                emit_qk_chunk(pr, c)
                if c >= 2:
                    emit_denom_chunk(pr, c - 2, 0)
                if c >= 3:
                    emit_denom_chunk(pr, c - 3, 1)
                for th in slots.get(c, ()):
                    th()
        # ---- tail ----
        last = n_pairs - 1
        emit_av_st(last - 1, 1, 3)   # group deferred past the boundary
        finish_prev(last)

        def wo_mm(ops, ec, st, fc, start, stop):
            nc.tensor.matmul(
                ops,
                lhsT=woT_sb[:, fc, ec * 128:(ec + 1) * 128],
                rhs=out_all[:, fc, st * NT:(st + 1) * NT],
                start=start, stop=stop,
                skip_group_check=True,
            )

        # head start: fc 0..2 of the first six Wo tiles don't depend on the
        # last pair at all -- computing those partials (into SBUF f32) keeps
        # the PE busy and HAM warm while the recip/scale chain (DVE + DMA
        # round trip) completes; the fc3 term is added after the AVs.
        order = [(st, ec) for st in range(NS) for ec in range(EC)]
        wpart = {}
        for st, ec in order[:6]:
            ops = psum.tile([128, NT], f32, tag="pqk", name=f"opsh_{ec}_{st}")
            for fc in range(n_pairs - 1):
                wo_mm(ops, ec, st, fc, fc == 0, fc == n_pairs - 2)
            wp = wostp.tile([128, NT], f32, tag="wpart", bufs=6,
                            name=f"wpart_{ec}_{st}")
            # ScalarE copy: ACT is idle after the last exp, and this keeps
            # the DVE queue clear for the recip chain's dstage copies
            nc.scalar.copy(out=wp, in_=ops)
            wpart[(st, ec)] = wp

        emit_av_st(last, 0)
        emit_av_st(last, 1)

        # ---- phase 4: partial Wo projection (recycles pqk PSUM slots) ----
        Add = mybir.AluOpType.add
        for st, ec in order:
            ops = psum.tile([128, NT], f32, tag="pqk", name=f"ops_{ec}_{st}")
            wost = wostp.tile([128, NT], mm_dt, tag="wost")
            if (st, ec) in wpart:
                wo_mm(ops, ec, st, n_pairs - 1, True, True)
                nc.vector.tensor_tensor(out=wost, in0=ops,
                                        in1=wpart[(st, ec)], op=Add)
            else:
                for fc in range(n_pairs):
                    wo_mm(ops, ec, st, fc, fc == 0, fc == n_pairs - 1)
                nc.vector.tensor_copy(out=wost, in_=ops)
            nc.sync.dma_start(
                out=out_part[ec * 128:(ec + 1) * 128,
                             st * NT:(st + 1) * NT],
                in_=wost)

    return nc


def make_in_maps(queries, keys, values, Wq, Wk, Wv, Wo, mode="fp16"):
    """Shard the full inputs into the 8 per-core input dicts."""
    queries = np.ascontiguousarray(queries, dtype=np.float32)
    keys = np.ascontiguousarray(keys, dtype=np.float32)
    values = np.ascontiguousarray(values, dtype=np.float32)
    Wq = np.asarray(Wq, dtype=np.float32)
    Wk = np.asarray(Wk, dtype=np.float32)
    Wv = np.asarray(Wv, dtype=np.float32)
    Wo = np.asarray(Wo, dtype=np.float32)
    WoT = np.ascontiguousarray(Wo.T)

    def blockdiag(W, head_base):
        blk = np.zeros((N_PAIRS_FULL, 128, 128), dtype=np.float32)
        for pr in range(N_PAIRS_FULL):
            h0 = head_base + 2 * pr
            blk[pr, :64, :64] = W[h0].T
            blk[pr, 64:, 64:] = W[h0 + 1].T
        return blk

    in_maps = []
    for c in range(N_CORES):
        b, hg = c // 2, c % 2
        r0, r1 = hg * 512, (hg + 1) * 512
        head_base = hg * HEADS_PER_CORE
        m = {
            "q_rows": np.ascontiguousarray(queries[b, r0:r1, :]),
            "k_rows": np.ascontiguousarray(keys[b, r0:r1, :]),
            "v_rows": np.ascontiguousarray(values[b, r0:r1, :]),
            "wqT": blockdiag(Wq, head_base),
            "wkT": blockdiag(Wk, head_base),
            "wvT": blockdiag(Wv, head_base),
            "woT": np.ascontiguousarray(WoT[r0:r1, :]),
        }
        if mode == "fp32r":
            m = {k: round_fp32r(v) for k, v in m.items()}
        elif mode == "fp16":
            m = {k: v.astype(np.float16) for k, v in m.items()}
        in_maps.append(m)
    return in_maps


def round_fp32r(a):
    """RNE with the low 12 mantissa bits dropped (TRN2 fp32r rounding)."""
    bits = np.ascontiguousarray(a, dtype=np.float32).view(np.uint32).astype(np.uint64)
    drop = 12
    mask = np.uint64((0xFFFFFFFF >> drop) << drop)
    half = np.uint64(1 << (drop - 1))
    lsb = (bits >> np.uint64(drop)) & np.uint64(1)
    rem = bits & np.uint64((1 << drop) - 1)
    up = (rem > half) | ((rem == half) & (lsb == 1))
    out = ((bits & mask) + np.where(up, np.uint64(1 << drop), np.uint64(0)))
    return out.astype(np.uint32).view(np.float32).reshape(np.asarray(a).shape)


LAST_RESULT = None


def kernel(queries, keys, values, Wq, Wk, Wv, Wo):
    """Full-input entry point: shard -> run on 8 NeuronCores -> unshard."""
    global LAST_RESULT
    from concourse.bass_utils import run_bass_kernel_spmd

    trace = bool(int(os.environ.get("BASS_KERNEL_TRACE", "0")))
    if trace:
        _install_ntff_shim()

    sbuf_dma = bool(int(os.environ.get("BASS_SBUF_DMA", "0")))
    nc = build_core_kernel(S=S_FULL, n_pairs=N_PAIRS_FULL, e_out=E,
                           mm_dt=mybir.dt.float16, e_dt=mybir.dt.float16,
                           sbuf_dma=sbuf_dma)
    in_maps = make_in_maps(queries, keys, values, Wq, Wk, Wv, Wo, mode="fp16")
    res = run_bass_kernel_spmd(nc, in_maps, core_ids=list(range(N_CORES)),
                               trace=trace)
    LAST_RESULT = res
    parts = [np.asarray(res.results[c]["out_part"], dtype=np.float32)
             for c in range(N_CORES)]
    out = np.empty((B, E, S_FULL), dtype=np.float32)
    for b in range(B):
        out[b] = parts[2 * b] + parts[2 * b + 1]
    return out


# revision 47
# speedup vs baseline: 1.0178x; 1.0178x over previous
"""Multi-head attention (nonstandard softmax normalization) on 8 Trainium2 cores.

Reference computation (B=4, E=1024, S=1024, H=16, HS=64):
  per (b, h):  q = Wq[h] @ Q_h,  k = Wk[h] @ K_h,  v = Wv[h] @ V_h   (feature-first [HS, S])
               pre[s,t] = q[:,s]. k[:,t] / 8
               e = exp(pre);  denom[t] = sum_u e[t,u];  post[s,t] = e[s,t] / denom[t]
               out_h = v @ post.T                                     ([HS, S])
  out = concat_h(out_h);  result[b] = Wo @ out[b]
Sharding: core c -> (b = c//2, head-group hg = c%2 of 8 heads); host sums the
two partial Wo products per batch.

The kernel is ACT(exp)-roofline bound: 8 heads x S^2 exps per core = 64
ACTIVATE instructions of N=1024 ~= 77us dense.  Everything is organized to
keep the exp stream dense:
 - per pair (2 heads), QK^T is computed transposed in [128,1024] fp32 PSUM
   tiles (tag "pqk", 3 bufs = 6 banks) so exp reads N=1024 chunks while the
   next chunk's matmuls run (pipeline depth 1.5 chunks).
 - the two heads' QK matmuls are K=64 row-tiles (base_partition 0/64) that
   run concurrently on the PE; AV uses col-tiled M=64 pairs.
 - the denominator (partition-dim sums of E) accumulates with lag-2 behind
   exp into ONE packed PSUM bank (h0 cols 0:256, h1 cols 256:512; per-element
   has_written semantics make the shared bank safe).
 - denom free-dim row -> per-partition recip via one strided DVE copy and a
   single SBUF->SBUF scatter DMA (no DRAM round trip), then one fused
   reciprocal + one broadcast multiply folds recip(denom) into v^T.
 - projections/v^T for pairs 1-3 are emitted as filler inside pair 0's QK
   stream so the first exp starts as early as possible.
 - the Wo phase recycles the (dead) "pqk" PSUM slots for dense accumulation.
"""

import os
import sys
import types

import numpy as np

import concourse.bass as bass
import concourse.mybir as mybir
import concourse.tile as tile
from contextlib import ExitStack

B, E, S_FULL, H = 4, 1024, 1024, 16
HS = 64
N_CORES = 8
HEADS_PER_CORE = H // 2          # 8: head-group per core
N_PAIRS_FULL = HEADS_PER_CORE // 2  # 4

_f32 = mybir.dt.float32


def _install_ntff_shim():
    """Register the axon NTFF profile hook if the image's antenv lacks it."""
    try:
        import antenv.axon_hooks  # noqa: F401
        return
    except ImportError:
        pass
    try:
        import antenv
        from trn_agent_boot.trn_boot import _ntff_profile_via_ctypes
    except ImportError:
        return
    mod = types.ModuleType("antenv.axon_hooks")
    mod._hook = None

    def set_axon_ntff_profile_hook(h):
        mod._hook = h

    def get_axon_ntff_profile_hook():
        return mod._hook

    mod.set_axon_ntff_profile_hook = set_axon_ntff_profile_hook
    mod.get_axon_ntff_profile_hook = get_axon_ntff_profile_hook
    sys.modules["antenv.axon_hooks"] = mod
    antenv.axon_hooks = mod
    for so in ("/opt/axon/libaxon_pjrt.so",):
        if os.path.exists(so):
            try:
                mod._hook = _ntff_profile_via_ctypes(so)
            except Exception:
                mod._hook = None
            break


def _install_drain_patch():
    """Work around this toolchain's walrus rejecting sem waits on Drain.

    TileContext's final drain carries end-of-kernel semaphore waits inline;
    this walrus build encodes Drain as NEURON_ISA_TPB_CTRL_NO_STRUCT and
    fails codegen ("Too many sync wait commands") for ANY inline wait.
    Equivalent semantics: emit the waits as standalone sync-engine wait
    instructions and leave the Drain bare.
    """
    if getattr(tile.TileContext, "_drain_patch_installed", False):
        return
    from concourse.vector_clock import ScopedClock

    def _patched_drain_and_barrier(self, tick_clock, wait_clock):
        drain_inst = self.nc.sync.drain()
        wait_clock.add_sem_waits(
            drain_inst.ins, ScopedClock({None: tick_clock.global_clock})
        )
        si = drain_inst.ins.sync_info
        waits = list(si.on_wait) if si is not None else []
        if waits:
            drain_inst.ins.sync_info = mybir.SyncInfo(
                on_wait=[], on_update=list(si.on_update) if si.on_update else []
            )
            by_name = (
                {h.name: h for h in self.sems.allocated().values()}
                if self.sems is not None else {}
            )
            for w in waits:
                sem = by_name.get(w.ant_name)
                assert sem is not None, f"unknown drain-wait sem: {w.ant_name}"
                assert w.wait_mode == "sem-ge-imm", w
                self.nc.sync.wait_ge(sem, w.wait_value)
        self.nc.all_engine_barrier()
        assert self.sems is not None
        popped = self.nc._tile_sem_poison_stack.pop()
        assert popped is self._sem_poison
        self.nc.clear_and_free_semaphores(list(self.sems.allocated().values()))
        self.nc.all_engine_barrier()

    tile.TileContext._drain_and_barrier = _patched_drain_and_barrier

    # Same walrus limitation, general form: at most ONE inline sem wait per
    # instruction; hoist all but the last onto EventSemaphore carriers.
    orig_add = tile.TileContext._add_instruction

    def _split_add_instruction(self, inst):
        si = inst.sync_info
        if si is not None and si.on_wait and len(si.on_wait) > 1:
            waits = list(si.on_wait)
            for w in waits[:-1]:
                ev = mybir.InstEventSemaphore(
                    name=self.nc.get_next_instruction_name(),
                    engine=inst.engine,
                    sync_info=mybir.SyncInfo(on_wait=[w], on_update=[]),
                )
                orig_add(self, ev)
            inst.sync_info = mybir.SyncInfo(
                on_wait=[waits[-1]],
                on_update=list(si.on_update) if si.on_update else [],
            )
        orig_add(self, inst)

    tile.TileContext._add_instruction = _split_add_instruction
    tile.TileContext._drain_patch_installed = True


def build_core_kernel(S=1024, n_pairs=4, e_out=1024, mm_dt=mybir.dt.float16,
                      e_dt=mybir.dt.float16, sbuf_dma=True):
    """Build the per-core Bass program (SPMD: same program on all cores)."""
    _install_drain_patch()
    C = S // 128            # t-chunks (chunk-contiguous: t = c*128 + p)
    NT = min(512, S)        # matmul moving free-dim tile
    NS = S // NT            # s-tiles (2)
    S4 = S // 4             # denom col-group width (256)
    EC = e_out // 128       # output e-chunks
    FP = n_pairs * 128      # feature rows handled by this core
    f32 = _f32
    in_dt = mm_dt

    nc = bass.Bass()
    q_rows = nc.declare_dram_parameter("q_rows", [FP, S], in_dt, isOutput=False)
    k_rows = nc.declare_dram_parameter("k_rows", [FP, S], in_dt, isOutput=False)
    v_rows = nc.declare_dram_parameter("v_rows", [FP, S], in_dt, isOutput=False)
    wqT = nc.declare_dram_parameter("wqT", [n_pairs, 128, 128], in_dt, isOutput=False)
    wkT = nc.declare_dram_parameter("wkT", [n_pairs, 128, 128], in_dt, isOutput=False)
    wvT = nc.declare_dram_parameter("wvT", [n_pairs, 128, 128], in_dt, isOutput=False)
    woT = nc.declare_dram_parameter("woT", [FP, e_out], in_dt, isOutput=False)
    out_part = nc.declare_dram_parameter("out_part", [e_out, S], mm_dt, isOutput=True)

    Exp = mybir.ActivationFunctionType.Exp
    Mult = mybir.AluOpType.mult

    with tile.TileContext(nc) as tc, ExitStack() as ctx:
        raws = ctx.enter_context(tc.tile_pool(name="raws", bufs=6))
        wop = ctx.enter_context(tc.tile_pool(name="wop", bufs=1))
        consts = ctx.enter_context(tc.tile_pool(name="consts", bufs=1))
        qks = ctx.enter_context(tc.tile_pool(name="qks", bufs=2))
        vts = ctx.enter_context(tc.tile_pool(name="vts", bufs=n_pairs))
        Epool = ctx.enter_context(tc.tile_pool(name="Epool", bufs=6))
        outp = ctx.enter_context(tc.tile_pool(name="outp", bufs=1))
        rcp = ctx.enter_context(tc.tile_pool(name="rcp", bufs=2))
        dstp = ctx.enter_context(tc.tile_pool(name="dstp", bufs=2))
        wostp = ctx.enter_context(tc.tile_pool(name="wostp", bufs=3))
        dram = ctx.enter_context(tc.tile_pool(name="dscratch", bufs=4, space="DRAM"))
        # one PSUM pool, 8 banks exactly:
        #   tag pqk : 3 x [128,1024] f32 = 6 banks (QK->exp pipeline)
        #   tag dps : 1 x [128, 512] f32 = 1 bank  (packed denominator)
        #   tag avp : 1 x [128, 512] f32 = 1 bank  (AV accum + transients)
        psum = ctx.enter_context(tc.tile_pool(name="psum", bufs=3, space="PSUM"))

        ones = consts.tile([128, 1], e_dt, tag="ones")
        nc.vector.memset(ones, 1.0)
        zeros = consts.tile([128, NT], mm_dt, tag="zeros")
        nc.vector.memset(zeros, 0.0)
        # HAM warm-up: dummy matmuls keep the PE busy while the first input
        # DMAs land, so the first real matmuls run at 2.4 GHz, not 1.2.
        warm_ps = psum.tile([128, NT], f32, tag="dps", bufs=1, name="warm_ps")
        for _ in range(12):
            nc.tensor.matmul(warm_ps, lhsT=zeros[:, :128], rhs=zeros,
                             start=True, stop=True)
        wq_sb = consts.tile([128, n_pairs, 128], in_dt, tag="wq")
        wk_sb = consts.tile([128, n_pairs, 128], in_dt, tag="wk")
        wv_sb = consts.tile([128, n_pairs, 128], in_dt, tag="wv")
        nc.sync.dma_start(out=wq_sb, in_=wqT.rearrange("r p m -> p r m"))
        nc.sync.dma_start(out=wk_sb, in_=wkT.rearrange("r p m -> p r m"))
        nc.sync.dma_start(out=wv_sb, in_=wvT.rearrange("r p m -> p r m"))

        q_all = qks.tile([128, n_pairs, S], mm_dt, tag="qall")
        k_all = qks.tile([128, n_pairs, S], mm_dt, tag="qall")
        out_all = outp.tile([128, n_pairs, S], mm_dt, tag="outall")

        vt_tiles = [None] * n_pairs
        raw_qk = {}
        raw_v = {}

        def load_qk(pr):
            qr = raws.tile([128, S], in_dt, tag="raw", name=f"qr_{pr}")
            kr = raws.tile([128, S], in_dt, tag="raw", name=f"kr_{pr}")
            for st in range(NS):
                sl = slice(st * NT, (st + 1) * NT)
                nc.sync.dma_start(out=qr[:, sl],
                                  in_=q_rows[pr * 128:(pr + 1) * 128, sl])
                nc.sync.dma_start(out=kr[:, sl],
                                  in_=k_rows[pr * 128:(pr + 1) * 128, sl])
            raw_qk[pr] = (qr, kr)

        def load_v(pr):
            vr = raws.tile([128, S], in_dt, tag="raw", name=f"vr_{pr}")
            nc.sync.dma_start(out=vr, in_=v_rows[pr * 128:(pr + 1) * 128, :])
            raw_v[pr] = vr

        _tcnt = [0]

        def trans_ps(name, tag="avp"):
            """Transient [128,512] PSUM tile; mid-stream transients share
            the 'avp' slot (pair 0's upfront proj can use free pqk slots)."""
            _tcnt[0] += 1
            bufs = None if tag == "pqk" else 1
            return psum.tile([128, NT], f32, tag=tag, bufs=bufs,
                             name=f"{name}_{_tcnt[0]}")

        def proj_qk(pr, which=None, tag="avp"):
            """Project q (which=0), k (which=1), or both into q_all/k_all."""
            qr, kr = raw_qk[pr]
            plan = ((qr, q_all, wq_sb), (kr, k_all, wk_sb))
            if which is not None:
                plan = (plan[which],)
            for src, dst, wt in plan:
                for st in range(NS):
                    ps = trans_ps(f"pj_{pr}_{st}", tag=tag)
                    nc.tensor.matmul(
                        ps,
                        lhsT=wt[:, pr, :],
                        rhs=src[:, st * NT:(st + 1) * NT],
                        start=True, stop=True,
                    )
                    nc.vector.tensor_copy(
                        out=dst[:, pr, st * NT:(st + 1) * NT], in_=ps)
            if which in (None, 1):
                raw_qk.pop(pr)

        def proj_v(pr, half):
            """v^T (transposed, two heads side by side) for 4 c-chunks."""
            vr = raw_v[pr]
            if vt_tiles[pr] is None:
                vt_tiles[pr] = vts.tile([128, C, 128], e_dt, tag="vt",
                                        name=f"vt_{pr}")
            vt = vt_tiles[pr]
            vrc = vr.rearrange("p (c t) -> p c t", c=C)
            c0 = half * (C // 2)
            ps = trans_ps(f"pv_{pr}_{half}")
            for j in range(C // 2):
                nc.tensor.matmul(
                    ps[:, j * 128:(j + 1) * 128],
                    lhsT=vrc[:, c0 + j, :],
                    rhs=wv_sb[:, pr, :],
                    start=True, stop=True,
                )
            nc.vector.tensor_copy(
                out=vt[:, c0:c0 + C // 2, :], in_=ps)
            if half == 1:
                raw_v.pop(pr)

        woT_sb = wop.tile([128, n_pairs, e_out], in_dt, tag="woT")

        # ---- per-pair phase-3 pieces ----
        Es_tiles = {}
        dps_tiles = {}

        def emit_qk_chunk(pr, c):
            kc = [
                k_all[64 * hh:64 * hh + 64, pr, :].rearrange(
                    "p (c t) -> p c t", c=C)
                for hh in (0, 1)
            ]
            pst = [psum.tile([128, S], f32, tag="pqk",
                             name=f"pqk_{pr}_{c}_{i}") for i in (0, 1)]
            def qk_mm(st, hh):
                nc.tensor.matmul(
                    pst[hh][:, st * NT:(st + 1) * NT],
                    lhsT=kc[hh][:, c, :],
                    rhs=q_all[64 * hh:64 * hh + 64, pr,
                              st * NT:(st + 1) * NT],
                    start=True, stop=True,
                    tile_position=(64 * hh, 0),
                    skip_group_check=True,
                )

            def qk_exp(hh):
                # E[t, s] = exp(preT[t, s] / 8)
                nc.scalar.activation(
                    out=Es_tiles[pr][hh][:, c, :], in_=pst[hh][:],
                    func=Exp, scale=0.125)

            qk_mm(0, 0)
            qk_mm(0, 1)
            qk_mm(1, 0)
            qk_mm(1, 1)
            qk_exp(0)
            qk_exp(1)

        def emit_denom_chunk(pr, c, hh):
            """Accumulate denom over chunk c into the packed dps bank.

            Only h0's c0 matmuls carry start=True: the hardware clear of the
            has_written bits is bank-wide, so a second start would wipe h0's
            accumulation.  h1's c0 (flags=0) overwrites because the bank-wide
            clear left its region's bits unset; the caller staggers h1 one
            chunk behind h0 so its writes cannot race the clear.
            """
            if pr not in dps_tiles:
                dps_tiles[pr] = psum.tile([128, NT], f32, tag="dps", bufs=1,
                                          name=f"dps_{pr}")
            dps = dps_tiles[pr]
            for q4 in range(4):
                nc.tensor.matmul(
                    dps[32 * q4:32 * q4 + 1, hh * S4:(hh + 1) * S4],
                    lhsT=ones,
                    rhs=Es_tiles[pr][hh][:, c, q4 * S4:(q4 + 1) * S4],
                    start=(c == 0 and hh == 0), stop=(c == C - 1),
                    tile_position=(0, 32 * q4),
                    skip_group_check=True,
                )

        def emit_recip_scale(pr):
            """denom rows -> per-partition recip, folded into v^T.

            Processed per head half so h0's DRAM round trip (and the vt
            scale of its half) starts one exp earlier than h1's.
            """
            dps = dps_tiles.pop(pr)
            dstage = dstp.tile([128, NT], f32, tag="dstage",
                               name=f"dstage_{pr}")
            rcrs = [rcp.tile([128, C], f32, tag=f"rcr{hh}",
                             name=f"rcr_{pr}_{hh}") for hh in (0, 1)]
            rcs = [rcp.tile([128, C], f32, tag=f"rc{hh}",
                            name=f"rc_{pr}_{hh}") for hh in (0, 1)]
            vt = vt_tiles[pr]
            for hh in (0, 1):
                sl = slice(hh * S4, (hh + 1) * S4)
                # full-partition copy (rows off the 32-grid are unread
                # garbage); one DVE op instead of four row copies
                nc.vector.tensor_copy(out=dstage[:, sl], in_=dps[:, sl])
                flat = dstage.rearrange("(a b) f -> a b f", b=32)[:, 0, :]
                scr = dram.tile([S], f32, tag=f"scr{hh}",
                                name=f"scr_{pr}_{hh}")
                nc.sync.dma_start(
                    out=scr.rearrange("(a f) -> a f", a=4),
                    in_=flat[:, sl])
                nc.sync.dma_start(
                    out=rcrs[hh],
                    in_=scr.rearrange("(c p) -> p c", p=128))
                nc.vector.reciprocal(out=rcs[hh], in_=rcrs[hh])
                nc.vector.tensor_tensor(
                    out=vt[:, :, 64 * hh:64 * hh + 64],
                    in0=vt[:, :, 64 * hh:64 * hh + 64],
                    in1=rcs[hh][:, :, None].to_broadcast((128, C, 64)),
                    op=Mult,
                )

        avp_tiles = {}

        def emit_av_st(pr, st, grp=None, tag="avp"):
            """AV accumulation; grp splits the 8 chunks into quarters so the
            PE burst per emission slot stays under ~1us."""
            key = (pr, st)
            if key not in avp_tiles:
                avp_tiles[key] = psum.tile([128, NT], f32, tag=tag, bufs=1,
                                           name=f"avp_{pr}_{st}")
            avp = avp_tiles[key]
            cr = range(C) if grp is None else range(
                grp * (C // 4), (grp + 1) * (C // 4))
            for c in cr:
                for hh in (0, 1):
                    nc.tensor.matmul(
                        avp[64 * hh:64 * hh + 64, :],
                        lhsT=vt_tiles[pr][:, c, 64 * hh:64 * hh + 64],
                        rhs=Es_tiles[pr][hh][:, c, st * NT:(st + 1) * NT],
                        start=(c == 0), stop=(c == C - 1),
                        tile_position=(0, 64 * hh),
                        skip_group_check=True,
                    )
            if grp is None or grp == 3:
                avp_tiles.pop(key)
                nc.vector.tensor_copy(
                    out=out_all[:, pr, st * NT:(st + 1) * NT], in_=avp)

        # ---- emission schedule ----
        # input DMAs for pair 0 first, then the rest (raws pool depth 6
        # naturally paces the prefetch)
        load_qk(0)
        load_v(0)
        load_qk(1)
        load_v(1)
        # the 1 MB woT transfer queues only after the startup-critical loads
        nc.sync.dma_start(out=woT_sb, in_=woT.rearrange("(f p) e -> p f e", p=128))
        load_qk(2)
        load_v(2)
        load_qk(3)
        load_v(3)

        proj_qk(0, tag="pqk")

        # per-stream filler thunks, indexed by chunk slot.  Each slot's PE
        # work is kept under ~1us so QK chunks (and thus the exp stream)
        # never fall behind; AV bursts are split in half and kept away from
        # the stream boundary.
        def finish_prev(p):
            # pair p's denom leftovers (h0 lag-2, h1 lag-3), then its
            # recip/scale; emitted one slot into the next stream so the
            # exps they wait on are already done when they hit the PE.
            emit_denom_chunk(p, C - 2, 0)
            emit_denom_chunk(p, C - 1, 0)
            emit_denom_chunk(p, C - 3, 1)
            emit_denom_chunk(p, C - 2, 1)
            emit_denom_chunk(p, C - 1, 1)
            emit_recip_scale(p)

        def stream_slots(pr):
            if pr == 0:
                return {
                    0: [lambda: proj_v(0, 0)],
                    1: [lambda: proj_v(0, 1)],
                    2: [lambda: proj_qk(1, 0)],
                    3: [lambda: proj_qk(1, 1)],
                    5: [lambda: proj_qk(2, 0)],
                    7: [lambda: proj_qk(2, 1)],
                }
            p = pr - 1
            slots = {
                0: ([lambda: emit_av_st(p - 1, 1, 3)] if pr >= 2 else []),
                1: [lambda: finish_prev(p)],
                2: [lambda: proj_v(pr, 0)],
                3: [lambda: proj_v(pr, 1)],
                4: [lambda: emit_av_st(p, 0, 0), lambda: emit_av_st(p, 0, 1)],
                5: [lambda: emit_av_st(p, 0, 2), lambda: emit_av_st(p, 0, 3)],
                6: [lambda: emit_av_st(p, 1, 0), lambda: emit_av_st(p, 1, 1)],
                7: [lambda: emit_av_st(p, 1, 2)],
            }
            if pr == 1:
                slots[7].append(lambda: proj_qk(3, 0))
            if pr == 2:
                slots[7].append(lambda: proj_qk(3, 1))
            return slots

        for pr in range(n_pairs):
            Es_tiles[pr] = (
                Epool.tile([128, C, S], e_dt, tag="E", name=f"E0_{pr}"),
                Epool.tile([128, C, S], e_dt, tag="E", name=f"E1_{pr}"),
            )
            slots = stream_slots(pr)
            for c in range(C):
                emit_qk_chunk(pr, c)
                if c >= 2:
                    emit_denom_chunk(pr, c - 2, 0)
                if c >= 3:
                    emit_denom_chunk(pr, c - 3, 1)
                for th in slots.get(c, ()):
                    th()
        # ---- tail ----
        last = n_pairs - 1
        emit_av_st(last - 1, 1, 3)   # group deferred past the boundary
        finish_prev(last)

        def wo_mm(ops, ec, st, fc, start, stop):
            nc.tensor.matmul(
                ops,
                lhsT=woT_sb[:, fc, ec * 128:(ec + 1) * 128],
                rhs=out_all[:, fc, st * NT:(st + 1) * NT],
                start=start, stop=stop,
                skip_group_check=True,
            )

        # head start: fc 0..2 of the first six Wo tiles don't depend on the
        # last pair at all -- computing those partials (into SBUF f32) keeps
        # the PE busy and HAM warm while the recip/scale chain (DVE + DMA
        # round trip) completes; the fc3 term is added after the AVs.
        order = [(st, ec) for st in range(NS) for ec in range(EC)]
        wpart = {}
        for st, ec in order[:6]:
            ops = psum.tile([128, NT], f32, tag="pqk", name=f"opsh_{ec}_{st}")
            for fc in range(n_pairs - 1):
                wo_mm(ops, ec, st, fc, fc == 0, fc == n_pairs - 2)
            wp = wostp.tile([128, NT], f32, tag="wpart", bufs=6,
                            name=f"wpart_{ec}_{st}")
            # ScalarE copy: ACT is idle after the last exp, and this keeps
            # the DVE queue clear for the recip chain's dstage copies
            nc.scalar.copy(out=wp, in_=ops)
            wpart[(st, ec)] = wp

        emit_av_st(last, 0)
        emit_av_st(last, 1)

        # ---- phase 4: partial Wo projection (recycles pqk PSUM slots) ----
        Add = mybir.AluOpType.add
        for st, ec in order:
            ops = psum.tile([128, NT], f32, tag="pqk", name=f"ops_{ec}_{st}")
            wost = wostp.tile([128, NT], mm_dt, tag="wost")
            if (st, ec) in wpart:
                wo_mm(ops, ec, st, n_pairs - 1, True, True)
                nc.vector.tensor_tensor(out=wost, in0=ops,
                                        in1=wpart[(st, ec)], op=Add)
            else:
                for fc in range(n_pairs):
                    wo_mm(ops, ec, st, fc, fc == 0, fc == n_pairs - 1)
                nc.vector.tensor_copy(out=wost, in_=ops)
            nc.sync.dma_start(
                out=out_part[ec * 128:(ec + 1) * 128,
                             st * NT:(st + 1) * NT],
                in_=wost)

    return nc


def make_in_maps(queries, keys, values, Wq, Wk, Wv, Wo, mode="fp16"):
    """Shard the full inputs into the 8 per-core input dicts."""
    queries = np.ascontiguousarray(queries, dtype=np.float32)
    keys = np.ascontiguousarray(keys, dtype=np.float32)
    values = np.ascontiguousarray(values, dtype=np.float32)
    Wq = np.asarray(Wq, dtype=np.float32)
    Wk = np.asarray(Wk, dtype=np.float32)
    Wv = np.asarray(Wv, dtype=np.float32)
    Wo = np.asarray(Wo, dtype=np.float32)
    WoT = np.ascontiguousarray(Wo.T)

    def blockdiag(W, head_base):
        blk = np.zeros((N_PAIRS_FULL, 128, 128), dtype=np.float32)
        for pr in range(N_PAIRS_FULL):
            h0 = head_base + 2 * pr
            blk[pr, :64, :64] = W[h0].T
            blk[pr, 64:, 64:] = W[h0 + 1].T
        return blk

    in_maps = []
    for c in range(N_CORES):
        b, hg = c // 2, c % 2
        r0, r1 = hg * 512, (hg + 1) * 512
        head_base = hg * HEADS_PER_CORE
        m = {
            "q_rows": np.ascontiguousarray(queries[b, r0:r1, :]),
            "k_rows": np.ascontiguousarray(keys[b, r0:r1, :]),
            "v_rows": np.ascontiguousarray(values[b, r0:r1, :]),
            "wqT": blockdiag(Wq, head_base),
            "wkT": blockdiag(Wk, head_base),
            "wvT": blockdiag(Wv, head_base),
            "woT": np.ascontiguousarray(WoT[r0:r1, :]),
        }
        if mode == "fp32r":
            m = {k: round_fp32r(v) for k, v in m.items()}
        elif mode == "fp16":
            m = {k: v.astype(np.float16) for k, v in m.items()}
        in_maps.append(m)
    return in_maps


def round_fp32r(a):
    """RNE with the low 12 mantissa bits dropped (TRN2 fp32r rounding)."""
    bits = np.ascontiguousarray(a, dtype=np.float32).view(np.uint32).astype(np.uint64)
    drop = 12
    mask = np.uint64((0xFFFFFFFF >> drop) << drop)
    half = np.uint64(1 << (drop - 1))
    lsb = (bits >> np.uint64(drop)) & np.uint64(1)
    rem = bits & np.uint64((1 << drop) - 1)
    up = (rem > half) | ((rem == half) & (lsb == 1))
    out = ((bits & mask) + np.where(up, np.uint64(1 << drop), np.uint64(0)))
    return out.astype(np.uint32).view(np.float32).reshape(np.asarray(a).shape)


LAST_RESULT = None


def kernel(queries, keys, values, Wq, Wk, Wv, Wo):
    """Full-input entry point: shard -> run on 8 NeuronCores -> unshard."""
    global LAST_RESULT
    from concourse.bass_utils import run_bass_kernel_spmd

    trace = bool(int(os.environ.get("BASS_KERNEL_TRACE", "0")))
    if trace:
        _install_ntff_shim()

    sbuf_dma = bool(int(os.environ.get("BASS_SBUF_DMA", "0")))
    nc = build_core_kernel(S=S_FULL, n_pairs=N_PAIRS_FULL, e_out=E,
                           mm_dt=mybir.dt.float16, e_dt=mybir.dt.float16,
                           sbuf_dma=sbuf_dma)
    in_maps = make_in_maps(queries, keys, values, Wq, Wk, Wv, Wo, mode="fp16")
    res = run_bass_kernel_spmd(nc, in_maps, core_ids=list(range(N_CORES)),
                               trace=trace)
    LAST_RESULT = res
    parts = [np.asarray(res.results[c]["out_part"], dtype=np.float32)
             for c in range(N_CORES)]
    out = np.empty((B, E, S_FULL), dtype=np.float32)
    for b in range(B):
        out[b] = parts[2 * b] + parts[2 * b + 1]
    return out


# revision 48
# speedup vs baseline: 1.1774x; 1.1569x over previous
"""Multi-head attention (nonstandard softmax normalization) on 8 Trainium2 cores.

Reference computation (B=4, E=1024, S=1024, H=16, HS=64):
  per (b, h):  q = Wq[h] @ Q_h,  k = Wk[h] @ K_h,  v = Wv[h] @ V_h   (feature-first [HS, S])
               pre[s,t] = q[:,s]. k[:,t] / 8
               e = exp(pre);  denom[t] = sum_u e[t,u];  post[s,t] = e[s,t] / denom[t]
               out_h = v @ post.T                                     ([HS, S])
  out = concat_h(out_h);  result[b] = Wo @ out[b]
Sharding: core c -> (b = c//2, head-group hg = c%2 of 8 heads); host sums the
two partial Wo products per batch.

The kernel is ACT(exp)-roofline bound: 8 heads x S^2 exps per core = 64
ACTIVATE instructions of N=1024 ~= 77us dense.  Everything is organized to
keep the exp stream dense:
 - per pair (2 heads), QK^T is computed transposed in [128,1024] fp32 PSUM
   tiles (tag "pqk", 3 bufs = 6 banks) so exp reads N=1024 chunks while the
   next chunk's matmuls run (pipeline depth 1.5 chunks).
 - the two heads' QK matmuls are K=64 row-tiles (base_partition 0/64) that
   run concurrently on the PE; AV uses col-tiled M=64 pairs.
 - the denominator (partition-dim sums of E) accumulates with lag-2 behind
   exp into ONE packed PSUM bank (h0 cols 0:256, h1 cols 256:512; per-element
   has_written semantics make the shared bank safe).
 - denom free-dim row -> per-partition recip via one strided DVE copy and a
   single SBUF->SBUF scatter DMA (no DRAM round trip), then one fused
   reciprocal + one broadcast multiply folds recip(denom) into v^T.
 - projections/v^T for pairs 1-3 are emitted as filler inside pair 0's QK
   stream so the first exp starts as early as possible.
 - the Wo phase recycles the (dead) "pqk" PSUM slots for dense accumulation.
"""

import os
import sys
import types

import numpy as np

import concourse.bass as bass
import concourse.mybir as mybir
import concourse.tile as tile
from contextlib import ExitStack

B, E, S_FULL, H = 4, 1024, 1024, 16
HS = 64
N_CORES = 8
HEADS_PER_CORE = H // 2          # 8: head-group per core
N_PAIRS_FULL = HEADS_PER_CORE // 2  # 4

_f32 = mybir.dt.float32


def _install_ntff_shim():
    """Register the axon NTFF profile hook if the image's antenv lacks it."""
    try:
        import antenv.axon_hooks  # noqa: F401
        return
    except ImportError:
        pass
    try:
        import antenv
        from trn_agent_boot.trn_boot import _ntff_profile_via_ctypes
    except ImportError:
        return
    mod = types.ModuleType("antenv.axon_hooks")
    mod._hook = None

    def set_axon_ntff_profile_hook(h):
        mod._hook = h

    def get_axon_ntff_profile_hook():
        return mod._hook

    mod.set_axon_ntff_profile_hook = set_axon_ntff_profile_hook
    mod.get_axon_ntff_profile_hook = get_axon_ntff_profile_hook
    sys.modules["antenv.axon_hooks"] = mod
    antenv.axon_hooks = mod
    for so in ("/opt/axon/libaxon_pjrt.so",):
        if os.path.exists(so):
            try:
                mod._hook = _ntff_profile_via_ctypes(so)
            except Exception:
                mod._hook = None
            break


def _install_drain_patch():
    """Work around this toolchain's walrus rejecting sem waits on Drain.

    TileContext's final drain carries end-of-kernel semaphore waits inline;
    this walrus build encodes Drain as NEURON_ISA_TPB_CTRL_NO_STRUCT and
    fails codegen ("Too many sync wait commands") for ANY inline wait.
    Equivalent semantics: emit the waits as standalone sync-engine wait
    instructions and leave the Drain bare.
    """
    if getattr(tile.TileContext, "_drain_patch_installed", False):
        return
    from concourse.vector_clock import ScopedClock

    def _patched_drain_and_barrier(self, tick_clock, wait_clock):
        drain_inst = self.nc.sync.drain()
        wait_clock.add_sem_waits(
            drain_inst.ins, ScopedClock({None: tick_clock.global_clock})
        )
        si = drain_inst.ins.sync_info
        waits = list(si.on_wait) if si is not None else []
        if waits:
            drain_inst.ins.sync_info = mybir.SyncInfo(
                on_wait=[], on_update=list(si.on_update) if si.on_update else []
            )
            by_name = (
                {h.name: h for h in self.sems.allocated().values()}
                if self.sems is not None else {}
            )
            for w in waits:
                sem = by_name.get(w.ant_name)
                assert sem is not None, f"unknown drain-wait sem: {w.ant_name}"
                assert w.wait_mode == "sem-ge-imm", w
                self.nc.sync.wait_ge(sem, w.wait_value)
        self.nc.all_engine_barrier()
        assert self.sems is not None
        popped = self.nc._tile_sem_poison_stack.pop()
        assert popped is self._sem_poison
        self.nc.clear_and_free_semaphores(list(self.sems.allocated().values()))
        self.nc.all_engine_barrier()

    tile.TileContext._drain_and_barrier = _patched_drain_and_barrier

    # Same walrus limitation, general form: at most ONE inline sem wait per
    # instruction; hoist all but the last onto EventSemaphore carriers.
    orig_add = tile.TileContext._add_instruction

    def _split_add_instruction(self, inst):
        si = inst.sync_info
        if si is not None and si.on_wait and len(si.on_wait) > 1:
            waits = list(si.on_wait)
            for w in waits[:-1]:
                ev = mybir.InstEventSemaphore(
                    name=self.nc.get_next_instruction_name(),
                    engine=inst.engine,
                    sync_info=mybir.SyncInfo(on_wait=[w], on_update=[]),
                )
                orig_add(self, ev)
            inst.sync_info = mybir.SyncInfo(
                on_wait=[waits[-1]],
                on_update=list(si.on_update) if si.on_update else [],
            )
        orig_add(self, inst)

    tile.TileContext._add_instruction = _split_add_instruction
    tile.TileContext._drain_patch_installed = True


def build_core_kernel(S=1024, n_pairs=4, e_out=1024, mm_dt=mybir.dt.float16,
                      e_dt=mybir.dt.float16, sbuf_dma=True):
    """Build the per-core Bass program (SPMD: same program on all cores)."""
    _install_drain_patch()
    C = S // 128            # t-chunks (chunk-contiguous: t = c*128 + p)
    NT = min(512, S)        # matmul moving free-dim tile
    NS = S // NT            # s-tiles (2)
    S4 = S // 4             # denom col-group width (256)
    EC = e_out // 128       # output e-chunks
    FP = n_pairs * 128      # feature rows handled by this core
    f32 = _f32
    in_dt = mm_dt

    nc = bass.Bass()
    q_rows = nc.declare_dram_parameter("q_rows", [FP, S], in_dt, isOutput=False)
    k_rows = nc.declare_dram_parameter("k_rows", [FP, S], in_dt, isOutput=False)
    v_rows = nc.declare_dram_parameter("v_rows", [FP, S], in_dt, isOutput=False)
    wqT = nc.declare_dram_parameter("wqT", [n_pairs, 128, 128], in_dt, isOutput=False)
    wkT = nc.declare_dram_parameter("wkT", [n_pairs, 128, 128], in_dt, isOutput=False)
    wvT = nc.declare_dram_parameter("wvT", [n_pairs, 128, 128], in_dt, isOutput=False)
    woT = nc.declare_dram_parameter("woT", [FP, e_out], in_dt, isOutput=False)
    out_part = nc.declare_dram_parameter("out_part", [e_out, S], mm_dt, isOutput=True)

    Exp = mybir.ActivationFunctionType.Exp
    Mult = mybir.AluOpType.mult

    with tile.TileContext(nc) as tc, ExitStack() as ctx:
        raws = ctx.enter_context(tc.tile_pool(name="raws", bufs=6))
        wop = ctx.enter_context(tc.tile_pool(name="wop", bufs=1))
        consts = ctx.enter_context(tc.tile_pool(name="consts", bufs=1))
        qks = ctx.enter_context(tc.tile_pool(name="qks", bufs=2))
        vts = ctx.enter_context(tc.tile_pool(name="vts", bufs=n_pairs))
        Epool = ctx.enter_context(tc.tile_pool(name="Epool", bufs=6))
        outp = ctx.enter_context(tc.tile_pool(name="outp", bufs=1))
        rcp = ctx.enter_context(tc.tile_pool(name="rcp", bufs=2))
        dstp = ctx.enter_context(tc.tile_pool(name="dstp", bufs=2))
        wostp = ctx.enter_context(tc.tile_pool(name="wostp", bufs=3))
        dram = ctx.enter_context(tc.tile_pool(name="dscratch", bufs=4, space="DRAM"))
        # one PSUM pool, 8 banks exactly:
        #   tag pqk : 3 x [128,1024] f32 = 6 banks (QK->exp pipeline)
        #   tag dps : 1 x [128, 512] f32 = 1 bank  (packed denominator)
        #   tag avp : 1 x [128, 512] f32 = 1 bank  (AV accum + transients)
        psum = ctx.enter_context(tc.tile_pool(name="psum", bufs=3, space="PSUM"))

        ones = consts.tile([128, 1], e_dt, tag="ones")
        nc.vector.memset(ones, 1.0)
        zeros = consts.tile([128, NT], mm_dt, tag="zeros")
        nc.vector.memset(zeros, 0.0)
        # HAM warm-up: dummy matmuls keep the PE busy while the first input
        # DMAs land, so the first real matmuls run at 2.4 GHz, not 1.2.
        warm_ps = psum.tile([128, NT], f32, tag="dps", bufs=1, name="warm_ps")
        for _ in range(12):
            nc.tensor.matmul(warm_ps, lhsT=zeros[:, :128], rhs=zeros,
                             start=True, stop=True)
        wq_sb = consts.tile([128, n_pairs, 128], in_dt, tag="wq")
        wk_sb = consts.tile([128, n_pairs, 128], in_dt, tag="wk")
        wv_sb = consts.tile([128, n_pairs, 128], in_dt, tag="wv")
        nc.sync.dma_start(out=wq_sb, in_=wqT.rearrange("r p m -> p r m"))
        nc.sync.dma_start(out=wk_sb, in_=wkT.rearrange("r p m -> p r m"))
        nc.sync.dma_start(out=wv_sb, in_=wvT.rearrange("r p m -> p r m"))

        q_all = qks.tile([128, n_pairs, S], mm_dt, tag="qall")
        k_all = qks.tile([128, n_pairs, S], mm_dt, tag="qall")
        out_all = outp.tile([128, n_pairs, S], mm_dt, tag="outall")

        vt_tiles = [None] * n_pairs
        raw_qk = {}
        raw_v = {}

        def load_qk(pr):
            qr = raws.tile([128, S], in_dt, tag="raw", name=f"qr_{pr}")
            kr = raws.tile([128, S], in_dt, tag="raw", name=f"kr_{pr}")
            for st in range(NS):
                sl = slice(st * NT, (st + 1) * NT)
                nc.sync.dma_start(out=qr[:, sl],
                                  in_=q_rows[pr * 128:(pr + 1) * 128, sl])
                nc.sync.dma_start(out=kr[:, sl],
                                  in_=k_rows[pr * 128:(pr + 1) * 128, sl])
            raw_qk[pr] = (qr, kr)

        def load_v(pr):
            vr = raws.tile([128, S], in_dt, tag="raw", name=f"vr_{pr}")
            nc.sync.dma_start(out=vr, in_=v_rows[pr * 128:(pr + 1) * 128, :])
            raw_v[pr] = vr

        _tcnt = [0]

        def trans_ps(name, tag="avp"):
            """Transient [128,512] PSUM tile; mid-stream transients share
            the 'avp' slot (pair 0's upfront proj can use free pqk slots)."""
            _tcnt[0] += 1
            bufs = None if tag == "pqk" else 1
            return psum.tile([128, NT], f32, tag=tag, bufs=bufs,
                             name=f"{name}_{_tcnt[0]}")

        def proj_qk(pr, which=None, tag="avp"):
            """Project q (which=0), k (which=1), or both into q_all/k_all."""
            qr, kr = raw_qk[pr]
            plan = ((qr, q_all, wq_sb), (kr, k_all, wk_sb))
            if which is not None:
                plan = (plan[which],)
            for src, dst, wt in plan:
                for st in range(NS):
                    ps = trans_ps(f"pj_{pr}_{st}", tag=tag)
                    nc.tensor.matmul(
                        ps,
                        lhsT=wt[:, pr, :],
                        rhs=src[:, st * NT:(st + 1) * NT],
                        start=True, stop=True,
                    )
                    nc.vector.tensor_copy(
                        out=dst[:, pr, st * NT:(st + 1) * NT], in_=ps)
            if which in (None, 1):
                raw_qk.pop(pr)

        def proj_v(pr, half):
            """v^T (transposed, two heads side by side) for 4 c-chunks."""
            vr = raw_v[pr]
            if vt_tiles[pr] is None:
                vt_tiles[pr] = vts.tile([128, C, 128], e_dt, tag="vt",
                                        name=f"vt_{pr}")
            vt = vt_tiles[pr]
            vrc = vr.rearrange("p (c t) -> p c t", c=C)
            c0 = half * (C // 2)
            ps = trans_ps(f"pv_{pr}_{half}")
            for j in range(C // 2):
                nc.tensor.matmul(
                    ps[:, j * 128:(j + 1) * 128],
                    lhsT=vrc[:, c0 + j, :],
                    rhs=wv_sb[:, pr, :],
                    start=True, stop=True,
                )
            nc.vector.tensor_copy(
                out=vt[:, c0:c0 + C // 2, :], in_=ps)
            if half == 1:
                raw_v.pop(pr)

        woT_sb = wop.tile([128, n_pairs, e_out], in_dt, tag="woT")

        # ---- per-pair phase-3 pieces ----
        Es_tiles = {}
        dps_tiles = {}

        def emit_qk_chunk(pr, c):
            kc = [
                k_all[64 * hh:64 * hh + 64, pr, :].rearrange(
                    "p (c t) -> p c t", c=C)
                for hh in (0, 1)
            ]
            pst = [psum.tile([128, S], f32, tag="pqk",
                             name=f"pqk_{pr}_{c}_{i}") for i in (0, 1)]
            def qk_mm(st, hh):
                nc.tensor.matmul(
                    pst[hh][:, st * NT:(st + 1) * NT],
                    lhsT=kc[hh][:, c, :],
                    rhs=q_all[64 * hh:64 * hh + 64, pr,
                              st * NT:(st + 1) * NT],
                    start=True, stop=True,
                    tile_position=(64 * hh, 0),
                    skip_group_check=True,
                )

            def qk_exp(hh):
                # E[t, s] = exp(preT[t, s] / 8)
                nc.scalar.activation(
                    out=Es_tiles[pr][hh][:, c, :], in_=pst[hh][:],
                    func=Exp, scale=0.125)

            qk_mm(0, 0)
            qk_mm(0, 1)
            qk_mm(1, 0)
            qk_mm(1, 1)
            qk_exp(0)
            qk_exp(1)

        def emit_denom_chunk(pr, c, hh):
            """Accumulate denom over chunk c into the packed dps bank.

            Only h0's c0 matmuls carry start=True: the hardware clear of the
            has_written bits is bank-wide, so a second start would wipe h0's
            accumulation.  h1's c0 (flags=0) overwrites because the bank-wide
            clear left its region's bits unset; the caller staggers h1 one
            chunk behind h0 so its writes cannot race the clear.
            """
            if pr not in dps_tiles:
                dps_tiles[pr] = psum.tile([128, NT], f32, tag="dps", bufs=1,
                                          name=f"dps_{pr}")
            dps = dps_tiles[pr]
            for q4 in range(4):
                nc.tensor.matmul(
                    dps[32 * q4:32 * q4 + 1, hh * S4:(hh + 1) * S4],
                    lhsT=ones,
                    rhs=Es_tiles[pr][hh][:, c, q4 * S4:(q4 + 1) * S4],
                    start=(c == 0 and hh == 0), stop=(c == C - 1),
                    tile_position=(0, 32 * q4),
                    skip_group_check=True,
                )

        def emit_recip_scale(pr):
            """denom rows -> per-partition recip, folded into v^T.

            Processed per head half so h0's DRAM round trip (and the vt
            scale of its half) starts one exp earlier than h1's.
            """
            dps = dps_tiles.pop(pr)
            dstage = dstp.tile([128, NT], f32, tag="dstage",
                               name=f"dstage_{pr}")
            rcrs = [rcp.tile([128, C], f32, tag=f"rcr{hh}",
                             name=f"rcr_{pr}_{hh}") for hh in (0, 1)]
            rcs = [rcp.tile([128, C], f32, tag=f"rc{hh}",
                            name=f"rc_{pr}_{hh}") for hh in (0, 1)]
            vt = vt_tiles[pr]
            for hh in (0, 1):
                sl = slice(hh * S4, (hh + 1) * S4)
                # full-partition copy (rows off the 32-grid are unread
                # garbage); one DVE op instead of four row copies
                nc.vector.tensor_copy(out=dstage[:, sl], in_=dps[:, sl])
                flat = dstage.rearrange("(a b) f -> a b f", b=32)[:, 0, :]
                scr = dram.tile([S], f32, tag=f"scr{hh}",
                                name=f"scr_{pr}_{hh}")
                nc.sync.dma_start(
                    out=scr.rearrange("(a f) -> a f", a=4),
                    in_=flat[:, sl])
                nc.sync.dma_start(
                    out=rcrs[hh],
                    in_=scr.rearrange("(c p) -> p c", p=128))
                nc.vector.reciprocal(out=rcs[hh], in_=rcrs[hh])
                nc.vector.tensor_tensor(
                    out=vt[:, :, 64 * hh:64 * hh + 64],
                    in0=vt[:, :, 64 * hh:64 * hh + 64],
                    in1=rcs[hh][:, :, None].to_broadcast((128, C, 64)),
                    op=Mult,
                )

        avp_tiles = {}

        def emit_av_st(pr, st, grp=None, tag="avp"):
            """AV accumulation; grp splits the 8 chunks into quarters so the
            PE burst per emission slot stays under ~1us."""
            key = (pr, st)
            if key not in avp_tiles:
                avp_tiles[key] = psum.tile([128, NT], f32, tag=tag, bufs=1,
                                           name=f"avp_{pr}_{st}")
            avp = avp_tiles[key]
            cr = range(C) if grp is None else range(
                grp * (C // 4), (grp + 1) * (C // 4))
            for c in cr:
                for hh in (0, 1):
                    nc.tensor.matmul(
                        avp[64 * hh:64 * hh + 64, :],
                        lhsT=vt_tiles[pr][:, c, 64 * hh:64 * hh + 64],
                        rhs=Es_tiles[pr][hh][:, c, st * NT:(st + 1) * NT],
                        start=(c == 0), stop=(c == C - 1),
                        tile_position=(0, 64 * hh),
                        skip_group_check=True,
                    )
            if grp is None or grp == 3:
                avp_tiles.pop(key)
                nc.vector.tensor_copy(
                    out=out_all[:, pr, st * NT:(st + 1) * NT], in_=avp)

        # ---- emission schedule ----
        # input DMAs for pair 0 first, then the rest (raws pool depth 6
        # naturally paces the prefetch)
        load_qk(0)
        load_v(0)
        load_qk(1)
        load_v(1)
        # the 1 MB woT transfer queues only after the startup-critical loads
        nc.sync.dma_start(out=woT_sb, in_=woT.rearrange("(f p) e -> p f e", p=128))
        load_qk(2)
        load_v(2)
        load_qk(3)
        load_v(3)

        proj_qk(0, tag="pqk")

        # per-stream filler thunks, indexed by chunk slot.  Each slot's PE
        # work is kept under ~1us so QK chunks (and thus the exp stream)
        # never fall behind; AV bursts are split in half and kept away from
        # the stream boundary.
        def finish_prev(p):
            # pair p's denom leftovers (h0 lag-2, h1 lag-3), then its
            # recip/scale; emitted one slot into the next stream so the
            # exps they wait on are already done when they hit the PE.
            emit_denom_chunk(p, C - 2, 0)
            emit_denom_chunk(p, C - 1, 0)
            emit_denom_chunk(p, C - 3, 1)
            emit_denom_chunk(p, C - 2, 1)
            emit_denom_chunk(p, C - 1, 1)
            emit_recip_scale(p)

        def stream_slots(pr):
            if pr == 0:
                # slots 0-1 carry no filler: the exp pipeline primes at
                # full rate through the first chunks
                return {
                    2: [lambda: proj_v(0, 0)],
                    3: [lambda: proj_v(0, 1)],
                    4: [lambda: proj_qk(1, 0)],
                    5: [lambda: proj_qk(1, 1)],
                    6: [lambda: proj_qk(2, 0)],
                    7: [lambda: proj_qk(2, 1)],
                }
            p = pr - 1
            slots = {
                0: ([lambda: emit_av_st(p - 1, 1, 3)] if pr >= 2 else []),
                1: [lambda: finish_prev(p)],
                2: [lambda: proj_v(pr, 0)],
                3: [lambda: proj_v(pr, 1)],
                4: [lambda: emit_av_st(p, 0, 0), lambda: emit_av_st(p, 0, 1)],
                5: [lambda: emit_av_st(p, 0, 2), lambda: emit_av_st(p, 0, 3)],
                6: [lambda: emit_av_st(p, 1, 0), lambda: emit_av_st(p, 1, 1)],
                7: [lambda: emit_av_st(p, 1, 2)],
            }
            if pr == 1:
                slots[7].append(lambda: proj_qk(3, 0))
            if pr == 2:
                slots[7].append(lambda: proj_qk(3, 1))
            return slots

        for pr in range(n_pairs):
            Es_tiles[pr] = (
                Epool.tile([128, C, S], e_dt, tag="E", name=f"E0_{pr}"),
                Epool.tile([128, C, S], e_dt, tag="E", name=f"E1_{pr}"),
            )
            slots = stream_slots(pr)
            for c in range(C):
                emit_qk_chunk(pr, c)
                if c >= 2:
                    emit_denom_chunk(pr, c - 2, 0)
                if c >= 3:
                    emit_denom_chunk(pr, c - 3, 1)
                for th in slots.get(c, ()):
                    th()
        # ---- tail ----
        last = n_pairs - 1
        emit_av_st(last - 1, 1, 3)   # group deferred past the boundary
        finish_prev(last)

        def wo_mm(ops, ec, st, fc, start, stop):
            nc.tensor.matmul(
                ops,
                lhsT=woT_sb[:, fc, ec * 128:(ec + 1) * 128],
                rhs=out_all[:, fc, st * NT:(st + 1) * NT],
                start=start, stop=stop,
                skip_group_check=True,
            )

        # head start: fc 0..2 of the first six Wo tiles don't depend on the
        # last pair at all -- computing those partials (into SBUF f32) keeps
        # the PE busy and HAM warm while the recip/scale chain (DVE + DMA
        # round trip) completes; the fc3 term is added after the AVs.
        order = [(st, ec) for st in range(NS) for ec in range(EC)]
        wpart = {}
        for st, ec in order[:6]:
            ops = psum.tile([128, NT], f32, tag="pqk", name=f"opsh_{ec}_{st}")
            for fc in range(n_pairs - 1):
                wo_mm(ops, ec, st, fc, fc == 0, fc == n_pairs - 2)
            wp = wostp.tile([128, NT], f32, tag="wpart", bufs=6,
                            name=f"wpart_{ec}_{st}")
            # ScalarE copy: ACT is idle after the last exp, and this keeps
            # the DVE queue clear for the recip chain's dstage copies
            nc.scalar.copy(out=wp, in_=ops)
            wpart[(st, ec)] = wp

        emit_av_st(last, 0)
        emit_av_st(last, 1)

        # ---- phase 4: partial Wo projection (recycles pqk PSUM slots) ----
        Add = mybir.AluOpType.add
        for st, ec in order:
            ops = psum.tile([128, NT], f32, tag="pqk", name=f"ops_{ec}_{st}")
            wost = wostp.tile([128, NT], mm_dt, tag="wost")
            if (st, ec) in wpart:
                wo_mm(ops, ec, st, n_pairs - 1, True, True)
                nc.vector.tensor_tensor(out=wost, in0=ops,
                                        in1=wpart[(st, ec)], op=Add)
            else:
                for fc in range(n_pairs):
                    wo_mm(ops, ec, st, fc, fc == 0, fc == n_pairs - 1)
                nc.vector.tensor_copy(out=wost, in_=ops)
            nc.sync.dma_start(
                out=out_part[ec * 128:(ec + 1) * 128,
                             st * NT:(st + 1) * NT],
                in_=wost)

    return nc


def make_in_maps(queries, keys, values, Wq, Wk, Wv, Wo, mode="fp16"):
    """Shard the full inputs into the 8 per-core input dicts."""
    queries = np.ascontiguousarray(queries, dtype=np.float32)
    keys = np.ascontiguousarray(keys, dtype=np.float32)
    values = np.ascontiguousarray(values, dtype=np.float32)
    Wq = np.asarray(Wq, dtype=np.float32)
    Wk = np.asarray(Wk, dtype=np.float32)
    Wv = np.asarray(Wv, dtype=np.float32)
    Wo = np.asarray(Wo, dtype=np.float32)
    WoT = np.ascontiguousarray(Wo.T)

    def blockdiag(W, head_base):
        blk = np.zeros((N_PAIRS_FULL, 128, 128), dtype=np.float32)
        for pr in range(N_PAIRS_FULL):
            h0 = head_base + 2 * pr
            blk[pr, :64, :64] = W[h0].T
            blk[pr, 64:, 64:] = W[h0 + 1].T
        return blk

    in_maps = []
    for c in range(N_CORES):
        b, hg = c // 2, c % 2
        r0, r1 = hg * 512, (hg + 1) * 512
        head_base = hg * HEADS_PER_CORE
        m = {
            "q_rows": np.ascontiguousarray(queries[b, r0:r1, :]),
            "k_rows": np.ascontiguousarray(keys[b, r0:r1, :]),
            "v_rows": np.ascontiguousarray(values[b, r0:r1, :]),
            "wqT": blockdiag(Wq, head_base),
            "wkT": blockdiag(Wk, head_base),
            "wvT": blockdiag(Wv, head_base),
            "woT": np.ascontiguousarray(WoT[r0:r1, :]),
        }
        if mode == "fp32r":
            m = {k: round_fp32r(v) for k, v in m.items()}
        elif mode == "fp16":
            m = {k: v.astype(np.float16) for k, v in m.items()}
        in_maps.append(m)
    return in_maps


def round_fp32r(a):
    """RNE with the low 12 mantissa bits dropped (TRN2 fp32r rounding)."""
    bits = np.ascontiguousarray(a, dtype=np.float32).view(np.uint32).astype(np.uint64)
    drop = 12
    mask = np.uint64((0xFFFFFFFF >> drop) << drop)
    half = np.uint64(1 << (drop - 1))
    lsb = (bits >> np.uint64(drop)) & np.uint64(1)
    rem = bits & np.uint64((1 << drop) - 1)
    up = (rem > half) | ((rem == half) & (lsb == 1))
    out = ((bits & mask) + np.where(up, np.uint64(1 << drop), np.uint64(0)))
    return out.astype(np.uint32).view(np.float32).reshape(np.asarray(a).shape)


LAST_RESULT = None


def kernel(queries, keys, values, Wq, Wk, Wv, Wo):
    """Full-input entry point: shard -> run on 8 NeuronCores -> unshard."""
    global LAST_RESULT
    from concourse.bass_utils import run_bass_kernel_spmd

    trace = bool(int(os.environ.get("BASS_KERNEL_TRACE", "0")))
    if trace:
        _install_ntff_shim()

    sbuf_dma = bool(int(os.environ.get("BASS_SBUF_DMA", "0")))
    nc = build_core_kernel(S=S_FULL, n_pairs=N_PAIRS_FULL, e_out=E,
                           mm_dt=mybir.dt.float16, e_dt=mybir.dt.float16,
                           sbuf_dma=sbuf_dma)
    in_maps = make_in_maps(queries, keys, values, Wq, Wk, Wv, Wo, mode="fp16")
    res = run_bass_kernel_spmd(nc, in_maps, core_ids=list(range(N_CORES)),
                               trace=trace)
    LAST_RESULT = res
    parts = [np.asarray(res.results[c]["out_part"], dtype=np.float32)
             for c in range(N_CORES)]
    out = np.empty((B, E, S_FULL), dtype=np.float32)
    for b in range(B):
        out[b] = parts[2 * b] + parts[2 * b + 1]
    return out
